# revision 2
# baseline (speedup 1.0000x reference)
"""MD-RNN (4-direction 2D GRU) Trainium2 kernel — fp8 DoubleRow edition.

Sharding: 8-way data-parallel over batch (B=256 -> 32 per core); each core runs
all 4 directional 2D-GRU scans as anti-diagonal wavefronts, interleaved so the
tensor engine stays busy while other engines run the gate nonlinearities.

Layout ("transposed" / hidden-on-partition):
  - hidden states stored as h^T tiles: [128, 2, cells*B] fp8e4 (dim1 = the two
    128-row chunks of H=256) — exactly the rhs shape DoubleRow matmuls want.
  - recurrent matmuls Uh^T@h_above / Uh2^T@h_left run in fp8 DoubleRow mode
    (2 k-tiles per instruction, ~1.5x bf16 throughput at fd>=256); small
    diagonals fall back to normal fp8 matmuls (FWL fast weight load).
  - the K=17 input-projection matmuls run as concurrent 32-row-strip matmuls
    (tile_position row tiling): 4 can stream through the PE simultaneously.
  - per chunk, psum tiles [128, 2, fd] collect r / z / (hn+h2n) / xn; gate
    math runs once per chunk over both H-chunks (halves instruction count).

The patch tensor (im2col of x + ones row for the bias trick, replicated at
partition offsets 0/32/64/96 for the strip matmuls) is built host-side (pure
data movement) and streamed per-diagonal from DRAM.
"""

import numpy as np
import ml_dtypes

GRID = 4
N_IMG = 32
S = N_IMG - (GRID - 1)          # 29 patch positions per axis
B_FULL = 256
N_CORES = 8
B = B_FULL // N_CORES           # 32 batch per core
H = 256
H3 = 3 * H                      # 768
OUT_DIM = 10
K_IN = GRID * GRID + 1          # 16 patch elems + ones row (bias trick)

FWD = list(range(S))                 # 29 entries
BWD = list(range(S - 2, -1, -1))     # 28 entries (reference off-by-one kept)
DIRS = [(FWD, FWD), (BWD, FWD), (FWD, BWD), (BWD, BWD)]

CELLS_PER_CHUNK = 16            # 16 cells * B=32 = 512 = one psum bank (fp32)
DR_MIN_FD = 256                 # DoubleRow only pays off at fd >= 256
STRIPS = [0, 1, 2, 3, 0, 1]     # PE row-strip per wx block (mc 0..5)
F8NP = ml_dtypes.float8_e4m3
REPEAT = 1                      # body repetitions (timing calibration only)


def _diag_infos():
    """Per direction: list over diagonals of (i_lo, i_hi, global cell base)."""
    infos = []
    base = 0
    for (yi, xi) in DIRS:
        ny, nx = len(yi), len(xi)
        diags = []
        for d in range(ny + nx - 1):
            ilo = max(0, d - (nx - 1))
            ihi = min(d, ny - 1)
            diags.append((ilo, ihi, base))
            base += ihi - ilo + 1
        infos.append(diags)
    return infos, base


DIAG_INFOS, TOT_CELLS = _diag_infos()


def _scan_index_arrays():
    """Image-space (y, x) of every cell in pt order (dir-major, diag-major)."""
    ys, xs = [], []
    for a, (yi, xi) in enumerate(DIRS):
        ny, nx = len(yi), len(xi)
        for d, (ilo, ihi, _) in enumerate(DIAG_INFOS[a]):
            for i in range(ilo, ihi + 1):
                ys.append(yi[i])
                xs.append(xi[d - i])
    return np.asarray(ys), np.asarray(xs)


YS, XS = _scan_index_arrays()


def _chunk_sizes(k):
    nch = (k + CELLS_PER_CHUNK - 1) // CELLS_PER_CHUNK
    lo = k // nch
    rem = k - lo * nch
    return [lo + 1] * rem + [lo] * (nch - rem)


def make_pt(xc):
    """(B, 32, 32) core batch slice -> [128, TOT_CELLS*B] fp8 patch matrix.

    Rows 32s..32s+17 hold the (patch | ones) block for PE row-strip s."""
    from numpy.lib.stride_tricks import sliding_window_view
    w = sliding_window_view(xc, (GRID, GRID), axis=(1, 2))   # (B, 29, 29, 4, 4)
    p = w[:, YS, XS].reshape(xc.shape[0], TOT_CELLS, GRID * GRID)  # (B, T, 16)
    p = np.ascontiguousarray(p.transpose(2, 1, 0)).reshape(GRID * GRID, -1)
    pt17 = np.concatenate([p, np.ones((1, p.shape[1]), np.float32)], axis=0)
    pt17 = pt17.astype(F8NP)
    pt = np.zeros((128, pt17.shape[1]), F8NP)
    for s in range(4):
        pt[32 * s:32 * s + K_IN] = pt17
    return pt


def make_weight_maps(Wx, Uh, Uh2, b, W_out, b_out):
    Wx, Uh, Uh2 = (np.asarray(t, np.float32) for t in (Wx, Uh, Uh2))
    b, W_out, b_out = (np.asarray(t, np.float32) for t in (b, W_out, b_out))
    # uh[a]: [128, mc, al, kt, 128] fp8 — DoubleRow weight pairs
    uh = np.zeros((4, 128, 6, 2, 2, 128), F8NP)
    for a in range(4):
        for mc in range(6):
            cols = slice(mc * 128, (mc + 1) * 128)
            for kt in range(2):
                rows = slice(kt * 128, (kt + 1) * 128)
                uh[a, :, mc, 0, kt, :] = Uh[a][rows, cols].astype(F8NP)
                uh[a, :, mc, 1, kt, :] = Uh2[a][rows, cols].astype(F8NP)
    # wx[a]: [128, block, 128] fp8 — (Wx|b) blocks at their strip partitions
    wx = np.zeros((4, 128, 6, 128), F8NP)
    for a in range(4):
        wxa = np.concatenate([Wx[a], b[a][None, :]], axis=0)   # (17, 768)
        for j in range(6):
            s = STRIPS[j]
            wx[a, 32 * s:32 * s + K_IN, j, :] = \
                wxa[:, j * 128:(j + 1) * 128].astype(F8NP)
    wo = np.ascontiguousarray(W_out.reshape(8, 128, OUT_DIM))
    bo = np.ascontiguousarray(b_out.reshape(1, OUT_DIM))
    return {"uh": uh, "wx": wx, "wo": wo, "bo": bo}


def _build_nc():
    import concourse.bacc as bacc
    import concourse.mybir as mybir
    import concourse.tile as tile

    f32 = mybir.dt.float32
    F8 = mybir.dt.float8e4
    BF = mybir.dt.bfloat16
    AF = mybir.ActivationFunctionType
    ALU = mybir.AluOpType
    DRM = mybir.MatmulPerfMode.DoubleRow

    nc = bacc.Bacc("TRN2", target_bir_lowering=False, debug=False,
                   num_devices=N_CORES)
    pt_d = nc.dram_tensor("pt", [128, TOT_CELLS * B], F8, kind="ExternalInput")
    uh_d = nc.dram_tensor("uh", [4, 128, 6, 2, 2, 128], F8,
                          kind="ExternalInput")
    wx_d = nc.dram_tensor("wx", [4, 128, 6, 128], F8, kind="ExternalInput")
    wo_d = nc.dram_tensor("wo", [8, 128, OUT_DIM], f32, kind="ExternalInput")
    bo_d = nc.dram_tensor("bo", [1, OUT_DIM], f32, kind="ExternalInput")
    out_d = nc.dram_tensor("out", [B, OUT_DIM], f32, kind="ExternalOutput")

    with tile.TileContext(nc) as tc:
        from contextlib import ExitStack
        with ExitStack() as ctx:
            const = ctx.enter_context(tc.tile_pool(name="const", bufs=1))
            ptp = ctx.enter_context(tc.tile_pool(name="ptp", bufs=8))
            ps = ctx.enter_context(tc.tile_pool(name="ps", bufs=1,
                                                space="PSUM"))
            hps = [ctx.enter_context(tc.tile_pool(name=f"h{a}", bufs=3))
                   for a in range(4)]
            ew = ctx.enter_context(tc.tile_pool(name="ew", bufs=3))
            hd = ctx.enter_context(tc.tile_pool(name="hd", bufs=1))

            # --- resident weights ---
            uh_sb, wx_sb = {}, {}
            for a in range(4):
                t = const.tile([128, 6, 2, 2, 128], F8, tag=f"uh{a}")
                nc.sync.dma_start(out=t, in_=uh_d[a])
                uh_sb[a] = t
                t2 = const.tile([128, 6, 128], F8, tag=f"wx{a}")
                nc.sync.dma_start(out=t2, in_=wx_d[a])
                wx_sb[a] = t2
            wo_sb = const.tile([128, 8 * OUT_DIM], f32, tag="wo")
            for c in range(8):
                nc.sync.dma_start(out=wo_sb[:, c * OUT_DIM:(c + 1) * OUT_DIM],
                                  in_=wo_d[c])
            bo_sb = const.tile([1, OUT_DIM], f32, tag="bo")
            nc.sync.dma_start(out=bo_sb, in_=bo_d[:, :])
            ones_sb = const.tile([1, B], f32, tag="ones")
            nc.vector.memset(ones_sb, 1.0)
            zero_h = const.tile([128, 2, 2 * B], F8, tag="zeroh")
            nc.vector.memset(zero_h, 0.0)

            def emit_chunk(a, prev_t, s_a, c0, c1, ht, ptd):
                fd = (c1 - c0) * B
                above = prev_t[:, :, (s_a + c0) * B:(s_a + c1) * B]
                left = prev_t[:, :, (s_a + 1 + c0) * B:(s_a + 1 + c1) * B]

                pr = ps.tile([128, 2, 512], f32, tag="r")
                pz = ps.tile([128, 2, 512], f32, tag="z")
                pn = ps.tile([128, 2, 512], f32, tag="nh")
                px = ps.tile([128, 2, 512], f32, tag="xn")

                # input projections: concurrent row-strip matmuls
                targets = [(pr, 0), (pr, 1), (pz, 0), (pz, 1), (px, 0),
                           (px, 1)]
                for j, (pt_, kc) in enumerate(targets):
                    s = STRIPS[j]
                    is_x = pt_ is px
                    nc.tensor.matmul(
                        pt_[:, kc, :fd],
                        wx_sb[a][32 * s:32 * s + K_IN, j, :],
                        ptd[32 * s:32 * s + K_IN, c0 * B:c1 * B],
                        start=True, stop=is_x, tile_position=(32 * s, 0))

                # recurrent matmuls
                if fd >= DR_MIN_FD:
                    for pt_, base_mc in ((pr, 0), (pz, 2), (pn, 4)):
                        for kc in (0, 1):
                            mc = base_mc + kc
                            nc.tensor.matmul(
                                pt_[:, kc, :fd], uh_sb[a][:, mc, 0], above,
                                start=(pt_ is pn), stop=False, perf_mode=DRM)
                            nc.tensor.matmul(
                                pt_[:, kc, :fd], uh_sb[a][:, mc, 1], left,
                                start=False, stop=True, perf_mode=DRM)
                else:
                    for pt_, base_mc in ((pr, 0), (pz, 2), (pn, 4)):
                        for kc in (0, 1):
                            mc = base_mc + kc
                            q = 0
                            for al in (0, 1):
                                src = above if al == 0 else left
                                for kt in (0, 1):
                                    nc.tensor.matmul(
                                        pt_[:, kc, :fd],
                                        uh_sb[a][:, mc, al, kt],
                                        src[:, kt, :],
                                        start=(pt_ is pn and q == 0),
                                        stop=(q == 3))
                                    q += 1

                # gate math, both H-chunks per op
                rt = ew.tile([128, 2, 512], BF, tag="rt")
                nc.scalar.activation(rt[:, :, :fd], pr[:, :, :fd], AF.Sigmoid)
                zt = ew.tile([128, 2, 512], BF, tag="zt")
                nc.scalar.activation(zt[:, :, :fd], pz[:, :, :fd], AF.Sigmoid)
                t1 = ew.tile([128, 2, 512], BF, tag="t1")
                nc.vector.tensor_mul(t1[:, :, :fd], rt[:, :, :fd],
                                     pn[:, :, :fd])
                t2 = ew.tile([128, 2, 512], BF, tag="t2")
                nc.vector.tensor_add(t2[:, :, :fd], t1[:, :, :fd],
                                     px[:, :, :fd])
                nt = ew.tile([128, 2, 512], BF, tag="nt")
                nc.scalar.activation(nt[:, :, :fd], t2[:, :, :fd], AF.Tanh)
                st = ew.tile([128, 2, 512], BF, tag="st")
                nc.gpsimd.tensor_add(st[:, :, :fd], above, left)
                dt_ = ew.tile([128, 2, 512], BF, tag="dt")
                nc.vector.scalar_tensor_tensor(
                    dt_[:, :, :fd], st[:, :, :fd], 0.5, nt[:, :, :fd],
                    ALU.mult, ALU.subtract)
                et = ew.tile([128, 2, 512], BF, tag="et")
                nc.vector.tensor_mul(et[:, :, :fd], zt[:, :, :fd],
                                     dt_[:, :, :fd])
                nc.gpsimd.tensor_add(ht[:, :, (1 + c0) * B:(1 + c1) * B],
                                     et[:, :, :fd], nt[:, :, :fd])

            # --- main wavefront, 4 directions interleaved per diagonal ---
            max_nd = max(len(di) for di in DIAG_INFOS)
            for _rep in range(REPEAT):
                h_prev = {a: None for a in range(4)}
                for d in range(max_nd):
                    pts = {}
                    for a in range(4):
                        if d >= len(DIAG_INFOS[a]):
                            continue
                        ilo, ihi, cbase = DIAG_INFOS[a][d]
                        k = ihi - ilo + 1
                        ptd = ptp.tile([128, S * B], F8, tag="pt")
                        nc.sync.dma_start(
                            out=ptd[:, :k * B],
                            in_=pt_d[:, cbase * B:(cbase + k) * B])
                        pts[a] = ptd
                    for a in range(4):
                        if d >= len(DIAG_INFOS[a]):
                            continue
                        ilo, ihi, cbase = DIAG_INFOS[a][d]
                        k = ihi - ilo + 1
                        ht = hps[a].tile([128, 2, (k + 2) * B], F8,
                                         tag=f"h{a}")
                        nc.vector.memset(ht[:, :, 0:B], 0.0)
                        nc.gpsimd.memset(ht[:, :, (k + 1) * B:(k + 2) * B],
                                         0.0)
                        if d == 0:
                            prev_t, k_prev, ilo_prev = zero_h, 0, 0
                        else:
                            prev_t, k_prev, ilo_prev = h_prev[a]
                        s_a = ilo - ilo_prev
                        assert 0 <= s_a and s_a + k <= k_prev + 2, (a, d)
                        c0 = 0
                        for cs in _chunk_sizes(k):
                            emit_chunk(a, prev_t, s_a, c0, c0 + cs, ht,
                                       pts[a])
                            c0 += cs
                        h_prev[a] = (ht, k, ilo)

            # --- head: logits = hcat @ W_out + b_out ; log_softmax ---
            hfin = []
            for a in range(4):
                ht, k, _ = h_prev[a]
                assert k == 1
                for kc in (0, 1):
                    t = hd.tile([128, B], f32, tag=f"hf{a}{kc}")
                    nc.scalar.copy(t, ht[:, kc, B:2 * B])
                    hfin.append(t)
            pl_t = ps.tile([128, 2, 512], f32, tag="r")
            pl = pl_t[:B, 0, :OUT_DIM]
            for c, t in enumerate(hfin):
                nc.tensor.matmul(pl, t, wo_sb[:, c * OUT_DIM:(c + 1) * OUT_DIM],
                                 start=(c == 0), stop=False)
            nc.tensor.matmul(pl, ones_sb[:1, :B], bo_sb, start=False,
                             stop=True)
            mx = hd.tile([B, 1], f32, tag="mx")
            nc.vector.reduce_max(mx, pl, axis=mybir.AxisListType.X)
            nmx = hd.tile([B, 1], f32, tag="nmx")
            nc.vector.tensor_scalar_mul(nmx, mx, -1.0)
            exv = hd.tile([B, OUT_DIM], f32, tag="exv")
            nc.scalar.activation(exv, pl, AF.Exp, bias=nmx, scale=1.0)
            sm = hd.tile([B, 1], f32, tag="sm")
            nc.vector.reduce_sum(sm, exv, axis=mybir.AxisListType.X)
            lnz = hd.tile([B, 1], f32, tag="lnz")
            nc.scalar.activation(lnz, sm, AF.Ln)
            tot = hd.tile([B, 1], f32, tag="tot")
            nc.vector.tensor_add(tot, lnz, mx)
            ntot = hd.tile([B, 1], f32, tag="ntot")
            nc.vector.tensor_scalar_mul(ntot, tot, -1.0)
            ot = hd.tile([B, OUT_DIM], f32, tag="ot")
            nc.scalar.activation(ot, pl, AF.Identity, bias=ntot, scale=1.0)
            nc.sync.dma_start(out=out_d[:, :], in_=ot)

    nc.compile()
    return nc


_CACHE = {}


def get_nc():
    if "nc" not in _CACHE:
        _CACHE["nc"] = _build_nc()
    return _CACHE["nc"]


def make_in_maps(x, Wx, Uh, Uh2, b, W_out, b_out):
    x = np.asarray(x, np.float32)
    wm = make_weight_maps(Wx, Uh, Uh2, b, W_out, b_out)
    in_maps = []
    for c in range(N_CORES):
        xc = x[c * B:(c + 1) * B]
        m = dict(wm)
        m["pt"] = make_pt(xc)
        in_maps.append(m)
    return in_maps


def kernel(x, Wx, Uh, Uh2, b, W_out, b_out):
    from concourse.bass_utils import run_bass_kernel_spmd
    nc = get_nc()
    in_maps = make_in_maps(x, Wx, Uh, Uh2, b, W_out, b_out)
    res = run_bass_kernel_spmd(nc, in_maps, list(range(N_CORES)))
    out = np.concatenate([res.results[c]["out"] for c in range(N_CORES)],
                         axis=0)
    return out.astype(np.float32)
